# revision 1
# baseline (speedup 1.0000x reference)
"""Multi-head causal self-attention on 8 Trainium2 NeuronCores.

Tensor-parallel over heads: core i owns heads (2i, 2i+1).
Per core:
  phase 1: qT/kT/vT = (W_slice^T @ x^T) for its 2 heads (xT provided by host);
           vT transposed on PE into [token, d] tiles for both heads at once.
  phase 2: per (b, h), k-chunk-outer loop sharing each stationary operand
           across all valid q-blocks: scoresT[k,q] = K Q^T -> exp ->
           (causal mask) -> out[d+1, q] accumulated as [V | 1]^T @ attnT
           (extra row = softmax denominator); normalize via 1/l broadcast.
  phase 3: partial projection P_i = W_proj[own rows]^T @ A_i, chunked over
           token ranges; ReduceScatter(add) each chunk across the 8 cores
           (overlaps with remaining attention compute); + bias.
Host reassembles the 8 column slices.

Matmuls run as float32r (fp32 storage, fast PE mode): ~230 ns per 512-col
matmul vs 429 ns for fp32, at ~1e-4 component relative error.
"""

import os

import numpy as np

B, T, C, H = 2, 2048, 1024, 16
D = C // H            # 64
NCORES = 8
HL = H // NCORES      # 2 heads per core
NT = B * T            # 4096
NQ = T // 512         # q-blocks of 512 per (b,h)
NK = T // 128         # k-chunks of 128 per (b,h)
SCALE = float(D) ** -0.5

MM_FAST = True        # float32r matmuls vs float32

_cache = {}


def _build(mode: str):
    """mode: 'causal' | 'none' (all-ones mask)."""
    import concourse.mybir as mybir
    import concourse.tile as tile
    from concourse import bacc

    f32 = mybir.dt.float32
    mdt = mybir.dt.float32r if MM_FAST else f32

    nc = bacc.Bacc("TRN2", target_bir_lowering=False, debug=False,
                   num_devices=NCORES)
    xT = nc.dram_tensor("xT", [C, NT], mdt, kind="ExternalInput").ap()
    wqkv = nc.dram_tensor("wqkv", [C, 3 * HL * D], mdt,
                          kind="ExternalInput").ap()
    wp = nc.dram_tensor("wp", [C, 128], mdt, kind="ExternalInput").ap()
    bias = nc.dram_tensor("bias", [128, 1], f32, kind="ExternalInput").ap()
    cmask = nc.dram_tensor("cmask", [128, 4 * 512 + 128], f32,
                           kind="ExternalInput").ap()
    ones_in = nc.dram_tensor("ones_in", [128, 64], mdt,
                             kind="ExternalInput").ap()
    outT = nc.dram_tensor("outT", [128, NT], f32, kind="ExternalOutput").ap()

    causal = mode == "causal"
    Exp = mybir.ActivationFunctionType.Exp

    with tile.TileContext(nc) as tc, \
         nc.allow_low_precision(reason="float32r matmul fast path"):
        with tc.tile_pool(name="persist", bufs=1) as persist, \
             tc.tile_pool(name="dram", bufs=1, space="DRAM") as dram:
            q_sb = persist.tile([128, NT], mdt)
            k_sb = persist.tile([128, NT], mdt)
            # V tiles, both heads, each with a trailing ones column:
            # cols 0:64 = head0 d, col 64 = ones, 65:129 = head1 d, 129 = ones
            vboth = persist.tile([128, 131, B * NK], mdt)
            a_sb = persist.tile([128, NT], mdt)   # normalized attn out
            cm_sb = persist.tile([128, 4 * 512 + 128], f32)
            ones_sb = persist.tile([1, 64], mdt)
            wqkv_sb = persist.tile([128, 8, 3 * HL * D], mdt)
            wp_sb = persist.tile([128, 8, 128], mdt)
            bias_sb = persist.tile([128, 1], f32)
            ag_in0 = dram.tile([128, T], mdt)
            ag_in1 = dram.tile([128, T], mdt)
            ag_out0 = dram.tile([C, T], mdt, addr_space="Shared")
            ag_out1 = dram.tile([C, T], mdt, addr_space="Shared")
            ag_ins = [ag_in0, ag_in1]
            ag_outs = [ag_out0, ag_out1]

            nc.sync.dma_start(out=cm_sb[:], in_=cmask[:])
            nc.sync.dma_start(out=wqkv_sb[:],
                              in_=wqkv.rearrange("(a p) n -> p a n", p=128))
            nc.sync.dma_start(out=wp_sb[:],
                              in_=wp.rearrange("(a p) n -> p a n", p=128))
            nc.sync.dma_start(out=bias_sb[:], in_=bias[:])
            nc.sync.dma_start(out=ones_sb[:], in_=ones_in[0:1, :])
            nc.sync.dma_start(out=vboth[:, 64, :], in_=ones_in[:, 0:B * NK])
            nc.sync.dma_start(out=vboth[:, 130 - 1, :],
                              in_=ones_in[:, 0:B * NK])
            ident = cm_sb[:, 4 * 512:4 * 512 + 128]

            # ---- phase 1: qkvT = W_slice^T @ xT ----
            with tc.tile_pool(name="xn_pool", bufs=6) as xp, \
                 tc.tile_pool(name="qkv_psum", bufs=4, space="PSUM") as pp, \
                 tc.tile_pool(name="vt_psum", bufs=2, space="PSUM") as vtp, \
                 tc.tile_pool(name="vtmp_pool", bufs=2) as vpool:
                for ng in range(2):
                    xns = []
                    for k in range(4):
                        n = ng * 4 + k
                        xn = xp.tile([128, 8, 512], mdt, tag="xn")
                        nc.sync.dma_start(
                            out=xn[:],
                            in_=xT.rearrange("(a p) t -> p a t", p=128)
                            [:, :, n * 512:(n + 1) * 512])
                        xns.append((n, xn))
                    for m in range(3):  # 0: q, 1: k, 2: v
                        pss = [pp.tile([128, 512], f32, tag="qkv", name="qkvp")
                               for _ in range(4)]
                        for kc in range(8):
                            for idx, (n, xn) in enumerate(xns):
                                nc.tensor.matmul(
                                    pss[idx][:],
                                    wqkv_sb[:, kc, m * 128:(m + 1) * 128],
                                    xn[:, kc, :],
                                    start=(kc == 0), stop=(kc == 7))
                        for idx, (n, xn) in enumerate(xns):
                            ps = pss[idx]
                            tok = slice(n * 512, (n + 1) * 512)
                            if m == 0:
                                nc.vector.tensor_copy(q_sb[:, tok], ps[:])
                            elif m == 1:
                                nc.vector.tensor_copy(k_sb[:, tok], ps[:])
                            else:
                                vtmp = vpool.tile([128, 512], f32)
                                nc.vector.tensor_copy(vtmp[:], ps[:])
                                b = n // NQ
                                for s in range(4):
                                    j = b * NK + (n % NQ) * 4 + s
                                    pt = vtp.tile([128, 128], f32)
                                    nc.tensor.transpose(
                                        pt[:],
                                        vtmp[:, s * 128:(s + 1) * 128],
                                        ident)
                                    nc.vector.tensor_copy(
                                        vboth[:, 0:64, j], pt[:, 0:64])
                                    nc.vector.tensor_copy(
                                        vboth[:, 65:129, j], pt[:, 64:128])

            # ---- phase 2 + 3: attention, then per-b proj + ReduceScatter ----
            with tc.tile_pool(name="s_psum", bufs=3, space="PSUM") as sp, \
                 tc.tile_pool(name="o_psum", bufs=4, space="PSUM") as op, \
                 tc.tile_pool(name="p_psum", bufs=1, space="PSUM") as prp, \
                 tc.tile_pool(name="attn_pool", bufs=6) as apool, \
                 tc.tile_pool(name="small_pool", bufs=2) as smp, \
                 tc.tile_pool(name="rb_pool", bufs=2) as rbp, \
                 tc.tile_pool(name="out_pool", bufs=2) as outp:
                for b in range(B):
                    for h in range(HL):
                        hs = slice(h * 64, (h + 1) * 64)
                        vcols = slice(65 * h, 65 * h + 65)
                        lrow = 64
                        drows = slice(0, 64)
                        po = [op.tile([65, 512], f32, tag="po", name="po")
                              for _ in range(NQ)]
                        for ki in range(NK):
                            qj0 = ki // 4 if causal else 0
                            ats = {}
                            for qj in range(qj0, NQ):
                                ps = sp.tile([128, 512], f32, tag="s")
                                nc.tensor.matmul(
                                    ps[:],
                                    k_sb[hs, b * T + ki * 128:
                                         b * T + (ki + 1) * 128],
                                    q_sb[hs, b * T + qj * 512:
                                         b * T + (qj + 1) * 512],
                                    start=True, stop=True)
                                at = apool.tile([128, 512], mdt, tag="at")
                                nc.scalar.activation(at[:], ps[:], Exp,
                                                     scale=SCALE)
                                if causal and qj == qj0:
                                    nc.vector.tensor_mul(
                                        at[:], at[:],
                                        cm_sb[:, (ki % 4) * 512:
                                              (ki % 4 + 1) * 512])
                                ats[qj] = at
                            for qj in range(qj0, NQ):
                                last = (4 * qj + 3) if causal else (NK - 1)
                                nc.tensor.matmul(
                                    po[qj][:], vboth[:, vcols, b * NK + ki],
                                    ats[qj][:],
                                    start=(ki == 0), stop=(ki == last))
                        # normalize: A = po[d] * (1/l) (l broadcast via PE)
                        lsb = smp.tile([1, 2048], f32)
                        for qj in range(NQ):
                            nc.vector.tensor_copy(
                                lsb[0:1, qj * 512:(qj + 1) * 512],
                                po[qj][lrow:lrow + 1, :])
                        ras = smp.tile([1, 2048], f32)
                        nc.vector.reciprocal_approx_fast(ras[:], lsb[:])
                        rl = smp.tile([1, 2048], mdt)
                        nc.vector.tensor_copy(rl[:], ras[:])
                        rb = rbp.tile([64, 2048], f32)
                        for qj in range(NQ):
                            pb = sp.tile([64, 512], f32, tag="s", name="pb")
                            nc.tensor.matmul(
                                pb[:], ones_sb[:],
                                rl[0:1, qj * 512:(qj + 1) * 512],
                                start=True, stop=True)
                            nc.vector.tensor_copy(
                                rb[:, qj * 512:(qj + 1) * 512], pb[:])
                        for qj in range(NQ):
                            tok = slice(b * T + qj * 512,
                                        b * T + (qj + 1) * 512)
                            nc.vector.tensor_mul(
                                a_sb[hs, tok], po[qj][drows, :],
                                rb[:, qj * 512:(qj + 1) * 512])
                    # stage this b's A slice for the AllGather
                    nc.sync.dma_start(out=ag_ins[b][:],
                                      in_=a_sb[:, b * T:(b + 1) * T])
                # ---- AllGather + local proj, per b (after all attention) ----
                for b in range(B):
                    t0 = b * T
                    nc.gpsimd.collective_compute(
                        "AllGather", mybir.AluOpType.bypass,
                        replica_groups=[list(range(NCORES))],
                        ins=[ag_ins[b].opt()], outs=[ag_outs[b].opt()])
                    for nn2 in range(4):
                        agt = outp.tile([128, 8, 512], mdt, tag="agt")
                        nc.sync.dma_start(
                            out=agt[:],
                            in_=ag_outs[b].rearrange("(a p) t -> p a t", p=128)
                            [:, :, nn2 * 512:(nn2 + 1) * 512])
                        pr = prp.tile([128, 512], f32, tag="pr")
                        for kc in range(8):
                            nc.tensor.matmul(
                                pr[:], wp_sb[:, kc, :], agt[:, kc, :],
                                start=(kc == 0), stop=(kc == 7))
                        ot = outp.tile([128, 512], f32, tag="ot")
                        nc.vector.tensor_scalar_add(ot[:], pr[:], bias_sb[:])
                        nc.sync.dma_start(
                            out=outT[:, t0 + nn2 * 512:t0 + (nn2 + 1) * 512],
                            in_=ot[:])

    nc.compile()
    return nc


def _get_program(mode: str):
    if mode not in _cache:
        _cache[mode] = _build(mode)
    return _cache[mode]


def kernel(**inputs):
    from concourse.bass_utils import run_bass_kernel_spmd

    x = np.ascontiguousarray(np.asarray(inputs["x"], dtype=np.float32))
    mask = np.asarray(inputs["causal_mask"])
    Wqkv = np.ascontiguousarray(np.asarray(inputs["W_qkv"], dtype=np.float32))
    Wp = np.ascontiguousarray(np.asarray(inputs["W_proj"], dtype=np.float32))
    bp = np.asarray(inputs["b_proj"], dtype=np.float32)

    m2 = np.asarray(mask).reshape(T, T)
    if np.all(m2 != 0):
        mode = "none"
    else:
        tril = np.tril(np.ones((T, T), dtype=m2.dtype))
        if np.array_equal(m2, tril):
            mode = "causal"
        else:
            raise NotImplementedError("general mask not supported")

    nc = _get_program(mode)

    xT = np.ascontiguousarray(x.reshape(NT, C).T)  # [C, NT]

    # causal-mask tile patterns (valid iff p <= f - 128*j) + 128x128 identity
    p = np.arange(128)[:, None]
    f = np.arange(512)[None, :]
    cm = np.concatenate(
        [(p <= f - 128 * j).astype(np.float32) for j in range(4)]
        + [np.eye(128, dtype=np.float32)], axis=1)

    Wq = Wqkv[:, 0 * C:1 * C]
    Wk = Wqkv[:, 1 * C:2 * C]
    Wv = Wqkv[:, 2 * C:3 * C]

    in_maps = []
    for i in range(NCORES):
        hcols = slice(2 * i * D, (2 * i + 2) * D)  # this core's 2 heads
        wqkv_i = np.concatenate(
            [Wq[:, hcols], Wk[:, hcols], Wv[:, hcols]], axis=1)  # [C, 384]
        in_maps.append({
            "xT": xT,
            "wqkv": np.ascontiguousarray(wqkv_i),
            "wp": np.ascontiguousarray(Wp[:, i * 128:(i + 1) * 128]),
            "bias": np.ascontiguousarray(bp[i * 128:(i + 1) * 128]
                                         .reshape(128, 1)),
            "cmask": cm,
            "ones_in": np.ones((128, 64), dtype=np.float32),
        })

    res = run_bass_kernel_spmd(nc, in_maps, list(range(NCORES)))

    out = np.empty((NT, C), dtype=np.float32)
    for i in range(NCORES):
        out[:, i * 128:(i + 1) * 128] = res.results[i]["outT"].T
    return out.reshape(B, T, C)



# revision 7
# speedup vs baseline: 1.6096x; 1.6096x over previous
"""Multi-head causal self-attention on 8 Trainium2 NeuronCores.

Tensor-parallel over heads: core i owns heads (2i, 2i+1). bf16 matmul
operands throughout (fp32 PSUM accumulation); tolerance is 2e-2.

Per core:
  phase 1: qT/kT/vT = (W_slice^T @ x^T) for its 2 heads; vT transposed on
           PE into [token, d] tiles (both heads + shared ones columns).
  phase 2: per (b, qj-block of 512 q, ki-chunk of 128 k), qj-outer:
           scoresT[k,q] for both heads packed as two row-tiled matmuls
           (head0 on PE rows 0-63, head1 on rows 64-127, concurrent);
           one Exp activation over the paired [128,1024] PSUM tile;
           causal-diagonal blocks narrowed to valid columns + [128,128]
           tril mask mul; PV accumulates [V_h | 1]^T @ attnT into
           po_h[65, 512] (row 64 = softmax denominator l).
           Per (b,h): copy po->araw (unnormalized + l), recip(l) on DVE,
           PE-broadcast 1/l, multiply into a_sb[128, 2048] (bf16).
  phase 3: per b: AllToAll shards a_sb by 256-token chunks, so core i
           ends up with [1024 features, 256 tokens] for tokens
           256i..256(i+1); local full W_proj^T @ A + bias -> outT chunk.
           A2A(b=0) overlaps b=1 attention.
Host reassembles the 8 token chunks per batch.
"""

import numpy as np

B, T, C, H = 2, 2048, 1024, 16
D = C // H            # 64
NCORES = 8
HL = H // NCORES      # 2 heads per core
NT = B * T            # 4096
NQ = T // 512         # 4 q-blocks of 512 per b
NK = T // 128         # 16 k-chunks of 128 per b
TCH = T // NCORES     # 256-token chunk per core per b (A2A shard)
SCALE = float(D) ** -0.5

_cache = {}


def _build(mode: str):
    """mode: 'causal' | 'none' (all-ones mask)."""
    import concourse.mybir as mybir
    import concourse.tile as tile
    from concourse import bacc

    f32 = mybir.dt.float32
    f32r = mybir.dt.float32r
    mdt = mybir.dt.bfloat16

    nc = bacc.Bacc("TRN2", target_bir_lowering=False, debug=False,
                   num_devices=NCORES)
    xT = nc.dram_tensor("xT", [C, NT], mdt, kind="ExternalInput").ap()
    wqkv = nc.dram_tensor("wqkv", [C, 3 * HL * D], mdt,
                          kind="ExternalInput").ap()
    wp = nc.dram_tensor("wp", [C, C], mdt, kind="ExternalInput").ap()
    bias = nc.dram_tensor("bias", [128, NCORES], f32,
                          kind="ExternalInput").ap()
    cmask = nc.dram_tensor("cmask", [128, 256], mdt,
                           kind="ExternalInput").ap()
    onesv = nc.dram_tensor("onesv", [128, B * NK], mdt,
                           kind="ExternalInput").ap()
    ones32 = nc.dram_tensor("ones32", [1, 64], f32,
                            kind="ExternalInput").ap()
    outT = nc.dram_tensor("outT", [C, B * TCH], f32,
                          kind="ExternalOutput").ap()

    causal = mode == "causal"
    Exp = mybir.ActivationFunctionType.Exp

    with tile.TileContext(nc) as tc, \
         nc.allow_low_precision(reason="bf16 matmul path, tol 2e-2"):
        with tc.tile_pool(name="persist", bufs=1) as persist, \
             tc.tile_pool(name="dram", bufs=1, space="DRAM") as dram:
            q_sb = persist.tile([128, NT], mdt)
            k_sb = persist.tile([128, NT], mdt)
            # V^T tiles: cols 0:64 head0 d, 64 ones, 65:129 head1 d,
            # 129 ones; lhsT slices [0:65] / [65:130] share nothing.
            vboth = persist.tile([128, 131, B * NK], mdt)
            cm_sb = persist.tile([128, 256], mdt)
            ones32_sb = persist.tile([1, 64], f32)
            wqkv_sb = persist.tile([128, 8, 3 * HL * D], mdt)
            wp_sb = persist.tile([128, 8, C], mdt)
            bias_sb = persist.tile([128, NCORES], f32)
            a2a_in0 = dram.tile([NCORES * 128, TCH], mdt)
            a2a_in1 = dram.tile([NCORES * 128, TCH], mdt)
            a2a_out0 = dram.tile([NCORES * 128, TCH], mdt)
            a2a_out1 = dram.tile([NCORES * 128, TCH], mdt)
            a2a_ins = [a2a_in0, a2a_in1]
            a2a_outs = [a2a_out0, a2a_out1]

            nc.sync.dma_start(out=wqkv_sb[:],
                              in_=wqkv.rearrange("(a p) n -> p a n", p=128))
            nc.gpsimd.dma_start(out=cm_sb[:], in_=cmask[:])
            nc.gpsimd.dma_start(out=ones32_sb[:], in_=ones32[:])
            nc.gpsimd.dma_start(out=bias_sb[:], in_=bias[:])
            nc.gpsimd.dma_start(out=vboth[:, 64, :], in_=onesv[:])
            nc.gpsimd.dma_start(out=vboth[:, 129, :], in_=onesv[:])
            nc.gpsimd.dma_start(out=wp_sb[:],
                                in_=wp.rearrange("(a p) n -> p a n", p=128))
            ident = cm_sb[:, 128:256]

            # PSUM layout (8 banks):
            #   mm1 (2 banks): phase-1 qkv ps + v-transposes + norm rb +
            #                  proj pr, all via shared slot group
            #   sc  (4 banks): paired score tiles [128,1024]
            #   po  (2 banks): po_h0 / po_h1 accumulators
            with tc.tile_pool(name="mm1", bufs=2, space="PSUM") as mm1, \
                 tc.tile_pool(name="sc_psum", bufs=2, space="PSUM") as scp, \
                 tc.tile_pool(name="po_psum", bufs=1, space="PSUM") as pop, \
                 tc.tile_pool(name="xn_pool", bufs=2) as xp, \
                 tc.tile_pool(name="vtmp_pool", bufs=2) as vpool, \
                 tc.tile_pool(name="at_pool", bufs=3) as apool, \
                 tc.tile_pool(name="araw_pool", bufs=2) as arp, \
                 tc.tile_pool(name="a_pool", bufs=2) as ap_pool, \
                 tc.tile_pool(name="smallf_pool", bufs=1) as smp, \
                 tc.tile_pool(name="agt_pool", bufs=2) as agp, \
                 tc.tile_pool(name="out_pool", bufs=3) as outp:

                def qkv_group(ng):
                    """QKV projection for token blocks ng*2048..(+2048)."""
                    xn = xp.tile([128, 8, 2048], mdt, tag="xn", name="xn")
                    for hh in range(2):
                        nc.sync.dma_start(
                            out=xn[:, :, hh * 1024:(hh + 1) * 1024],
                            in_=xT.rearrange("(a p) t -> p a t", p=128)
                            [:, :, ng * 2048 + hh * 1024:
                             ng * 2048 + (hh + 1) * 1024])
                    for nl in range(4):
                        n = ng * 4 + nl
                        tok = slice(n * 512, (n + 1) * 512)
                        for m in range(3):  # 0: q, 1: k, 2: v
                            ps = mm1.tile([128, 512], f32, tag="ps",
                                          name="ps")
                            for kc in range(8):
                                nc.tensor.matmul(
                                    ps[:],
                                    wqkv_sb[:, kc, m * 128:(m + 1) * 128],
                                    xn[:, kc, nl * 512:(nl + 1) * 512],
                                    start=(kc == 0), stop=(kc == 7))
                            if m == 0:
                                nc.vector.tensor_copy(q_sb[:, tok], ps[:])
                            elif m == 1:
                                nc.vector.tensor_copy(k_sb[:, tok], ps[:])
                            else:
                                vtmp = vpool.tile([128, 512], mdt,
                                                  tag="vtmp", name="vtmp")
                                nc.vector.tensor_copy(vtmp[:], ps[:])
                                bb = n // NQ
                                for s in range(4):
                                    j = bb * NK + (n % NQ) * 4 + s
                                    pt = mm1.tile([128, 128], mdt, tag="ps",
                                                  name="pt")
                                    nc.tensor.transpose(
                                        pt[:],
                                        vtmp[:, s * 128:(s + 1) * 128],
                                        ident)
                                    nc.vector.tensor_copy(
                                        vboth[:, 0:64, j], pt[:, 0:64])
                                    nc.vector.tensor_copy(
                                        vboth[:, 65:129, j], pt[:, 64:128])

                def attention_b(b):
                    """Attention for batch b -> a_sb [128, 2048] bf16."""
                    a_sb = ap_pool.tile([128, T], mdt, tag="a_sb",
                                        name="a_sb")
                    araws = [arp.tile([65, T], mdt, tag=f"araw{h}",
                                      name=f"araw{h}") for h in range(2)]
                    for qj in range(NQ):
                        last_ki = 4 * qj + 3 if causal else NK - 1
                        po0 = pop.tile([65, 512], f32, tag="po0",
                                       name="po0")
                        po1 = pop.tile([65, 512], f32, tag="po1",
                                       name="po1")
                        pos = [po0, po1]
                        for ki in range(last_ki + 1):
                            diag = causal and ki >= 4 * qj
                            st = (ki - 4 * qj) * 128 if diag else 0
                            kc_ = slice(b * T + ki * 128,
                                        b * T + (ki + 1) * 128)
                            qc = slice(b * T + qj * 512 + st,
                                       b * T + (qj + 1) * 512)
                            sc = scp.tile([128, 1024], f32, tag="sc",
                                          name="sc")
                            nc.tensor.matmul(
                                sc[:, st:512], k_sb[0:64, kc_],
                                q_sb[0:64, qc], start=True, stop=True)
                            nc.tensor.matmul(
                                sc[:, 512 + st:1024], k_sb[64:128, kc_],
                                q_sb[64:128, qc], start=True, stop=True)
                            at = apool.tile([128, 1024], mdt, tag="at",
                                            name="at")
                            if diag:
                                nc.scalar.activation(
                                    at[:, st:512], sc[:, st:512], Exp,
                                    scale=SCALE)
                                nc.scalar.activation(
                                    at[:, 512 + st:1024],
                                    sc[:, 512 + st:1024], Exp, scale=SCALE)
                                nc.vector.tensor_mul(
                                    at[:, st:st + 128], at[:, st:st + 128],
                                    cm_sb[:, 0:128])
                                nc.vector.tensor_mul(
                                    at[:, 512 + st:512 + st + 128],
                                    at[:, 512 + st:512 + st + 128],
                                    cm_sb[:, 0:128])
                            else:
                                nc.scalar.activation(at[:], sc[:], Exp,
                                                     scale=SCALE)
                            vj = b * NK + ki
                            for h in range(2):
                                nc.tensor.matmul(
                                    pos[h][:, st:512],
                                    vboth[:, 65 * h:65 * h + 65, vj],
                                    at[:, 512 * h + st:512 * h + 512],
                                    start=(ki == 0), stop=(ki == last_ki))
                        for h in range(2):
                            nc.vector.tensor_copy(
                                araws[h][:, qj * 512:(qj + 1) * 512],
                                pos[h][:])
                    # normalize: a = araw[0:64] * (1/l), l = araw[64]
                    for h in range(2):
                        lsb = smp.tile([1, T], f32, tag="lsb", name="lsb")
                        nc.vector.tensor_copy(lsb[:], araws[h][64:65, :])
                        ras = smp.tile([1, T], f32, tag="ras", name="ras")
                        nc.vector.reciprocal_approx_fast(ras[:], lsb[:])
                        for qj in range(NQ):
                            rb = mm1.tile([64, 512], f32, tag="ps",
                                          name="rb")
                            nc.tensor.matmul(
                                rb[:], ones32_sb[:],
                                ras[0:1, qj * 512:(qj + 1) * 512],
                                start=True, stop=True)
                            nc.vector.tensor_mul(
                                a_sb[64 * h:64 * h + 64,
                                     qj * 512:(qj + 1) * 512],
                                araws[h][0:64, qj * 512:(qj + 1) * 512],
                                rb[:])
                    nc.sync.dma_start(
                        out=a2a_ins[b].rearrange("(c p) t -> p c t", p=128),
                        in_=a_sb[:].rearrange("p (c t) -> p c t",
                                              c=NCORES))
                    return a_sb

                def proj_b(b):
                    """A2A + local projection for batch b's token chunk."""
                    nc.gpsimd.collective_compute(
                        "AllToAll", mybir.AluOpType.bypass,
                        replica_groups=[list(range(NCORES))],
                        ins=[a2a_ins[b].opt()], outs=[a2a_outs[b].opt()])
                    agt = agp.tile([128, 8, TCH], mdt, tag="agt",
                                   name="agt")
                    nc.sync.dma_start(
                        out=agt[:],
                        in_=a2a_outs[b].rearrange("(c p) t -> p c t",
                                                  p=128))
                    for o in range(8):
                        pr = mm1.tile([128, TCH], f32, tag="ps", name="pr")
                        for kc in range(8):
                            nc.tensor.matmul(
                                pr[:], wp_sb[:, kc, o * 128:(o + 1) * 128],
                                agt[:, kc, :],
                                start=(kc == 0), stop=(kc == 7))
                        ot = outp.tile([128, TCH], f32, tag="ot", name="ot")
                        nc.vector.tensor_scalar_add(ot[:], pr[:],
                                                    bias_sb[:, o:o + 1])
                        nc.sync.dma_start(
                            out=outT[o * 128:(o + 1) * 128,
                                     b * TCH:(b + 1) * TCH],
                            in_=ot[:])

                qkv_group(0)
                qkv_group(1)
                attention_b(0)
                attention_b(1)
                proj_b(0)
                proj_b(1)

    nc.compile()
    return nc


def _get_program(mode: str):
    if mode not in _cache:
        _cache[mode] = _build(mode)
    return _cache[mode]


def kernel(**inputs):
    import ml_dtypes
    from concourse.bass_utils import run_bass_kernel_spmd

    bf16 = ml_dtypes.bfloat16

    x = np.asarray(inputs["x"], dtype=np.float32)
    mask = np.asarray(inputs["causal_mask"])
    Wqkv = np.asarray(inputs["W_qkv"], dtype=np.float32)
    Wp = np.asarray(inputs["W_proj"], dtype=np.float32)
    bp = np.asarray(inputs["b_proj"], dtype=np.float32)

    m2 = mask.reshape(T, T)
    if np.all(m2 != 0):
        mode = "none"
    else:
        tril = np.tril(np.ones((T, T), dtype=m2.dtype))
        if np.array_equal(m2, tril):
            mode = "causal"
        else:
            raise NotImplementedError("general mask not supported")

    nc = _get_program(mode)

    xT = np.ascontiguousarray(x.reshape(NT, C).T).astype(bf16)

    # [128,256]: triu mask (k<=q within a diagonal 128-block) | identity
    p = np.arange(128)[:, None]
    f = np.arange(128)[None, :]
    cm = np.concatenate(
        [(p <= f).astype(np.float32), np.eye(128, dtype=np.float32)],
        axis=1).astype(bf16)

    Wq = Wqkv[:, 0 * C:1 * C]
    Wk = Wqkv[:, 1 * C:2 * C]
    Wv = Wqkv[:, 2 * C:3 * C]
    wp_bf = np.ascontiguousarray(Wp).astype(bf16)
    bias_h = np.ascontiguousarray(bp.reshape(NCORES, 128).T)

    in_maps = []
    for i in range(NCORES):
        hcols = slice(2 * i * D, (2 * i + 2) * D)  # this core's 2 heads
        wqkv_i = np.concatenate(
            [Wq[:, hcols], Wk[:, hcols], Wv[:, hcols]], axis=1)
        in_maps.append({
            "xT": xT,
            "wqkv": np.ascontiguousarray(wqkv_i).astype(bf16),
            "wp": wp_bf,
            "bias": bias_h,
            "cmask": cm,
            "onesv": np.ones((128, B * NK), dtype=bf16),
            "ones32": np.ones((1, 64), dtype=np.float32),
        })

    res = run_bass_kernel_spmd(nc, in_maps, list(range(NCORES)))

    out = np.empty((B, T, C), dtype=np.float32)
    for i in range(NCORES):
        oT = res.results[i]["outT"]  # [C, B*TCH] f32
        for b in range(B):
            out[b, i * TCH:(i + 1) * TCH, :] = \
                oT[:, b * TCH:(b + 1) * TCH].T
    return out
